# revision 1
# baseline (speedup 1.0000x reference)
"""GCNII (8 layers, N=50000, E=800000) on 8 trn2 NeuronCores.

Sharding: nodes partitioned into 8 contiguous ranges (6250/core); edges
partitioned by destination so each core owns the scatter-add for its node
range. Per layer: AllGather h -> HBM h_full; each core dma_gathers the
source rows for its edges, scatters them into PSUM via one-hot matmuls
(128-dst windows, norm folded into the one-hot), then applies the dense
epilogue with the layer matrix folded as M = (1-beta)I + beta*W on host.
"""
import numpy as np
import concourse.bass as bass
import concourse.mybir as mybir
from concourse import bacc, tile
from concourse.bass_utils import run_bass_kernel_spmd

mdt = mybir.dt

N = 50000
E = 800000
FIN = 128
HID = 64
L = 8
ALPHA = 0.1
THETA = 0.5
NCORES = 8
NS = N // NCORES            # 6250 nodes per core
NW = (NS + 127) // 128      # 49 windows per core
NSPAD = NW * 128            # 6272
CW = 7                      # windows per chunk
CHUNKS = NW // CW           # 7 chunks
assert CW * CHUNKS == NW
HALF = 32768                # int16 gather index split


def _preprocess(x, edge_index, w_in, b_in, conv_w, w_out, b_out):
    row = np.asarray(edge_index[0], dtype=np.int64)
    col = np.asarray(edge_index[1], dtype=np.int64)
    loops = np.arange(N, dtype=np.int64)
    row = np.concatenate([row, loops])
    col = np.concatenate([col, loops])
    deg = np.bincount(col, minlength=N).astype(np.float32)
    dinv = (1.0 / np.sqrt(deg)).astype(np.float32)
    norm = (dinv[row] * dinv[col]).astype(np.float32)

    per_core = []
    for c in range(NCORES):
        m = (col >= c * NS) & (col < (c + 1) * NS)
        r = row[m]
        d = col[m] - c * NS
        nv = norm[m]
        o = np.argsort(d, kind="stable")
        per_core.append((r[o], d[o], nv[o]))

    # per-window lo/hi counts; tile counts shared across cores
    counts = np.zeros((NCORES, NW, 2), dtype=np.int64)
    bounds = []
    for c in range(NCORES):
        r, d, nv = per_core[c]
        wb = np.searchsorted(d, np.arange(0, NSPAD + 1, 128))
        bounds.append(wb)
        for w in range(NW):
            seg = slice(wb[w], wb[w + 1])
            nlo = int((r[seg] < HALF).sum())
            counts[c, w] = (nlo, (wb[w + 1] - wb[w]) - nlo)
    TLs = np.maximum(np.ceil(counts[:, :, 0].max(axis=0) / 128), 1).astype(np.int64)
    THs = np.maximum(np.ceil(counts[:, :, 1].max(axis=0) / 128), 1).astype(np.int64)

    # global tile order: chunks; within chunk all lo tiles (window-major)
    # then all hi tiles (window-major)
    gidx_lo = np.zeros(NW, dtype=np.int64)
    gidx_hi = np.zeros(NW, dtype=np.int64)
    g = 0
    for s in range(CHUNKS):
        for wi in range(CW):
            w = s * CW + wi
            gidx_lo[w] = g
            g += TLs[w]
        for wi in range(CW):
            w = s * CW + wi
            gidx_hi[w] = g
            g += THs[w]
    T = g

    dstw = np.full((NCORES, 128, T), -1.0, dtype=np.float32)
    nrm = np.zeros((NCORES, 128, T), dtype=np.float32)
    idx16 = np.zeros((NCORES, 128, 8 * T), dtype=np.int16)
    for c in range(NCORES):
        r, d, nv = per_core[c]
        wb = bounds[c]
        for w in range(NW):
            seg = slice(wb[w], wb[w + 1])
            rs, ds, ns_ = r[seg], d[seg], nv[seg]
            mlo = rs < HALF
            for p in range(2):
                mask = mlo if p == 0 else ~mlo
                rr = rs[mask] - (0 if p == 0 else HALF)
                dd = ds[mask] - w * 128
                nn = ns_[mask]
                TT = int(TLs[w] if p == 0 else THs[w])
                g0 = int(gidx_lo[w] if p == 0 else gidx_hi[w])
                cap = TT * 128
                rrp = np.zeros(cap, np.int64)
                rrp[: len(rr)] = rr
                ddp = np.full(cap, -1.0, np.float32)
                ddp[: len(dd)] = dd
                nnp = np.zeros(cap, np.float32)
                nnp[: len(nn)] = nn
                for t in range(TT):
                    gg = g0 + t
                    dstw[c, :, gg] = ddp[t * 128 : (t + 1) * 128]
                    nrm[c, :, gg] = nnp[t * 128 : (t + 1) * 128]
                    v = rrp[t * 128 : (t + 1) * 128].astype(np.int16)
                    idx16[c, :, 8 * gg : 8 * (gg + 1)] = np.tile(
                        v.reshape(8, 16).T, (8, 1)
                    )

    # dense weights (shared across cores)
    w_in = np.asarray(w_in, np.float32)
    conv_w = np.asarray(conv_w, np.float32)
    w_out = np.asarray(w_out, np.float32)
    b_in = np.asarray(b_in, np.float32)
    b_out = np.asarray(b_out, np.float32)
    betas = np.log(THETA / np.arange(1, L + 1, dtype=np.float32) + 1.0)
    convT = np.concatenate(
        [
            ((1.0 - betas[l]) * np.eye(HID, dtype=np.float32) + betas[l] * conv_w[l]).T
            for l in range(L)
        ],
        axis=1,
    )  # [64, L*64]
    consts = {
        "w_inT": np.ascontiguousarray(w_in.T),              # [128, 64]
        "convT": np.ascontiguousarray(convT),               # [64, 512]
        "w_outT": np.ascontiguousarray(w_out.T),            # [64, 64]
        "b_in_rep": np.tile(b_in[None, :], (128, 1)),       # [128, 64]
        "b_out_rep": np.tile(b_out[None, :], (128, 1)),     # [128, 64]
        "iota": np.tile(np.arange(128, dtype=np.float32), (128, 1)),
        "ident": np.eye(128, dtype=np.float32),
    }

    x = np.asarray(x, np.float32)
    in_maps = []
    for c in range(NCORES):
        xT = np.zeros((FIN, NSPAD), np.float32)
        xT[:, :NS] = x[c * NS : (c + 1) * NS].T
        in_maps.append(
            dict(
                consts,
                xT=np.ascontiguousarray(xT),
                idx16=np.ascontiguousarray(idx16[c]),
                dstw=np.ascontiguousarray(dstw[c]),
                nrm=np.ascontiguousarray(nrm[c]),
            )
        )
    return in_maps, TLs, THs, gidx_lo, gidx_hi, T


def _build(TLs, THs, gidx_lo, gidx_hi, T, reps=1, sim_single=False):
    nc = bacc.Bacc(None, target_bir_lowering=False, num_devices=NCORES, num_swdge_queues=4)

    xT_in = nc.dram_tensor("xT", [FIN, NSPAD], mdt.float32, kind="ExternalInput")
    idx_in = nc.dram_tensor("idx16", [128, 8 * T], mdt.int16, kind="ExternalInput")
    dstw_in = nc.dram_tensor("dstw", [128, T], mdt.float32, kind="ExternalInput")
    nrm_in = nc.dram_tensor("nrm", [128, T], mdt.float32, kind="ExternalInput")
    w_inT_in = nc.dram_tensor("w_inT", [FIN, HID], mdt.float32, kind="ExternalInput")
    convT_in = nc.dram_tensor("convT", [HID, L * HID], mdt.float32, kind="ExternalInput")
    w_outT_in = nc.dram_tensor("w_outT", [HID, HID], mdt.float32, kind="ExternalInput")
    b_in_in = nc.dram_tensor("b_in_rep", [128, HID], mdt.float32, kind="ExternalInput")
    b_out_in = nc.dram_tensor("b_out_rep", [128, HID], mdt.float32, kind="ExternalInput")
    iota_in = nc.dram_tensor("iota", [128, 128], mdt.float32, kind="ExternalInput")
    ident_in = nc.dram_tensor("ident", [128, 128], mdt.float32, kind="ExternalInput")

    out_t = nc.dram_tensor("out", [NS, HID], mdt.float32, kind="ExternalOutput")

    bounce = nc.dram_tensor("bounce", [NS, HID], mdt.float32)
    h_full = nc.dram_tensor("h_full", [N, HID], mdt.float32, addr_space="Shared")

    # per-chunk sizes and offsets
    NLO = [int(TLs[s * CW : (s + 1) * CW].sum()) for s in range(CHUNKS)]
    NHI = [int(THs[s * CW : (s + 1) * CW].sum()) for s in range(CHUNKS)]
    MAXLO, MAXHI = max(NLO), max(NHI)

    with tile.TileContext(nc) as tc, \
         tc.tile_pool(name="const", bufs=1) as cpool, \
         tc.tile_pool(name="gath", bufs=2) as gpool, \
         tc.tile_pool(name="oh", bufs=4) as ohpool, \
         tc.tile_pool(name="work", bufs=3) as wpool, \
         tc.tile_pool(name="ps_sc", bufs=2, space="PSUM") as psum_sc, \
         tc.tile_pool(name="ps_tr", bufs=2, space="PSUM") as psum_tr, \
         tc.tile_pool(name="ps_mm", bufs=2, space="PSUM") as psum_mm:

        # ---- persistent constants ----
        iota_t = cpool.tile([128, 128], mdt.float32)
        nc.sync.dma_start(iota_t[:], iota_in[:])
        ident_t = cpool.tile([128, 128], mdt.float32)
        nc.sync.dma_start(ident_t[:], ident_in[:])
        w_inT_t = cpool.tile([FIN, HID], mdt.float32)
        nc.sync.dma_start(w_inT_t[:], w_inT_in[:])
        convT_t = cpool.tile([HID, L * HID], mdt.float32)
        nc.sync.dma_start(convT_t[:], convT_in[:])
        w_outT_t = cpool.tile([HID, HID], mdt.float32)
        nc.sync.dma_start(w_outT_t[:], w_outT_in[:])
        b_in_t = cpool.tile([128, HID], mdt.float32)
        nc.sync.dma_start(b_in_t[:], b_in_in[:])
        b_out_t = cpool.tile([128, HID], mdt.float32)
        nc.sync.dma_start(b_out_t[:], b_out_in[:])
        idx_t = cpool.tile([128, 8 * T], mdt.int16)
        nc.sync.dma_start(idx_t[:], idx_in[:])
        dstw_t = cpool.tile([128, T], mdt.float32)
        nc.sync.dma_start(dstw_t[:], dstw_in[:])
        nrm_t = cpool.tile([128, T], mdt.float32)
        nc.sync.dma_start(nrm_t[:], nrm_in[:])
        xT_t = cpool.tile([FIN, NSPAD], mdt.float32)
        nc.sync.dma_start(xT_t[:], xT_in[:])

        h_sb = cpool.tile([128, NW * HID], mdt.float32)
        x0s = cpool.tile([128, NW * HID], mdt.float32)

        def store_h(w):
            nrows = min(NS - w * 128, 128)
            nc.sync.dma_start(
                bounce[w * 128 : w * 128 + nrows, :],
                h_sb[:nrows, w * HID : (w + 1) * HID],
            )

        for rep_i in range(reps):
            # ---- h0 = relu(x @ w_in.T + b_in); x0s = ALPHA * h0 ----
            for w in range(NW):
                ps = psum_mm.tile([128, HID], mdt.float32, tag="mm")
                nc.tensor.matmul(
                    ps[:], xT_t[:, w * 128 : (w + 1) * 128], w_inT_t[:],
                    start=True, stop=True,
                )
                hw = h_sb[:, w * HID : (w + 1) * HID]
                u = wpool.tile([128, HID], mdt.float32, tag="u")
                nc.vector.tensor_tensor(u[:], ps[:], b_in_t[:], mybir.AluOpType.add)
                nc.scalar.activation(hw, u[:], mybir.ActivationFunctionType.Relu)
                nc.vector.tensor_scalar_mul(x0s[:, w * HID : (w + 1) * HID], hw, ALPHA)
                store_h(w)

            def allgather():
                if sim_single:
                    nc.sync.dma_start(h_full[:NS, :], bounce[:])
                else:
                    nc.gpsimd.collective_compute(
                        "AllGather", mybir.AluOpType.bypass,
                        replica_groups=[list(range(NCORES))],
                        ins=[bounce[:]], outs=[h_full[:]],
                    )

            allgather()

            # ---- layers ----
            qctr = [0]
            for l in range(L):
                for s in range(CHUNKS):
                    nlo, nhi = NLO[s], NHI[s]
                    base = int(gidx_lo[s * CW])  # first tile of this chunk
                    glo = gpool.tile([128, MAXLO, HID], mdt.float32, tag="glo")
                    ghi = gpool.tile([128, MAXHI, HID], mdt.float32, tag="ghi")

                    def one_gather(dst, src_ap, tile0, ntiles, nsplit=8):
                        bnds = [tile0 + (ntiles * i) // nsplit for i in range(nsplit + 1)]
                        for i in range(nsplit):
                            a, b = bnds[i], bnds[i + 1]
                            if a == b:
                                continue
                            nidx = (b - a) * 128
                            nc.gpsimd.dma_gather(
                                dst[:, a - tile0 : b - tile0, :], src_ap,
                                idx_t[:, 8 * a : 8 * b], nidx, nidx, HID,
                                single_packet=False, queue_num=qctr[0] % 4,
                            )
                            qctr[0] += 1

                    one_gather(glo, h_full[:, :], base, nlo)
                    one_gather(ghi, h_full[HALF:, :], base + nlo, nhi)
                    for wi in range(CW):
                        w = s * CW + wi
                        ntiles = int(TLs[w] + THs[w])
                        ps = psum_sc.tile([128, HID], mdt.float32, tag="sc")
                        k = 0
                        for p in range(2):
                            TT = int(TLs[w] if p == 0 else THs[w])
                            g0 = int(gidx_lo[w] if p == 0 else gidx_hi[w])
                            gsrc = glo if p == 0 else ghi
                            for t in range(TT):
                                gg = g0 + t
                                slot = gg - base if p == 0 else gg - base - nlo
                                oh = ohpool.tile([128, 128], mdt.float32, tag="oh")
                                nc.vector.tensor_scalar(
                                    oh[:], iota_t[:],
                                    dstw_t[:, gg : gg + 1], nrm_t[:, gg : gg + 1],
                                    mybir.AluOpType.is_equal, mybir.AluOpType.mult,
                                )
                                nc.tensor.matmul(
                                    ps[:], oh[:], gsrc[:, slot, :],
                                    start=(k == 0), stop=(k == ntiles - 1),
                                )
                                k += 1
                        # z = 0.9 * ps + x0s ; h = relu(z @ M_l.T)
                        zw = wpool.tile([128, HID], mdt.float32, tag="zw")
                        nc.vector.scalar_tensor_tensor(
                            zw[:], ps[:], 1.0 - ALPHA, x0s[:, w * HID : (w + 1) * HID],
                            mybir.AluOpType.mult, mybir.AluOpType.add,
                        )
                        zt_ps = psum_tr.tile([HID, 128], mdt.float32, tag="tr")
                        nc.tensor.transpose(zt_ps[:], zw[:], ident_t[:])
                        zt = wpool.tile([HID, 128], mdt.float32, tag="zt")
                        nc.scalar.copy(zt[:], zt_ps[:])
                        ps2 = psum_mm.tile([128, HID], mdt.float32, tag="mm")
                        nc.tensor.matmul(
                            ps2[:], zt[:], convT_t[:, l * HID : (l + 1) * HID],
                            start=True, stop=True,
                        )
                        hw = h_sb[:, w * HID : (w + 1) * HID]
                        nc.scalar.activation(hw, ps2[:], mybir.ActivationFunctionType.Relu)
                        if l < L - 1:
                            store_h(w)
                if l < L - 1:
                    allgather()

            # ---- out = h @ w_out.T + b_out ----
            for w in range(NW):
                ht_ps = psum_tr.tile([HID, 128], mdt.float32, tag="tr")
                nc.tensor.transpose(ht_ps[:], h_sb[:, w * HID : (w + 1) * HID], ident_t[:])
                ht = wpool.tile([HID, 128], mdt.float32, tag="zt")
                nc.scalar.copy(ht[:], ht_ps[:])
                ps3 = psum_mm.tile([128, HID], mdt.float32, tag="mm")
                nc.tensor.matmul(ps3[:], ht[:], w_outT_t[:], start=True, stop=True)
                ow = wpool.tile([128, HID], mdt.float32, tag="ow")
                nc.vector.tensor_tensor(ow[:], ps3[:], b_out_t[:], mybir.AluOpType.add)
                nrows = min(NS - w * 128, 128)
                nc.sync.dma_start(out_t[w * 128 : w * 128 + nrows, :], ow[:nrows, :])

    nc.finalize()
    return nc


def kernel(**inputs) -> np.ndarray:
    in_maps, TLs, THs, gidx_lo, gidx_hi, T = _preprocess(
        inputs["x"], inputs["edge_index"], inputs["w_in"], inputs["b_in"],
        inputs["conv_w"], inputs["w_out"], inputs["b_out"],
    )
    nc = _build(TLs, THs, gidx_lo, gidx_hi, T)
    res = run_bass_kernel_spmd(nc, in_maps, list(range(NCORES)))
    out = np.concatenate([res.results[c]["out"] for c in range(NCORES)], axis=0)
    return out



# revision 14
# speedup vs baseline: 1.5383x; 1.5383x over previous
"""GCNII (8 layers, N=50000, E=800000) on 8 trn2 NeuronCores.

V3: feature-transposed compute + fp16 paired-row gather. Nodes are
partitioned into 8 contiguous ranges (6250/core); edges partitioned by
destination so each core owns the scatter-add for its node range.

All dense math runs on h^T [64feat x nodes] so the scatter matmul
(lhsT=gathered tile, rhs=one-hot) accumulates z^T directly in PSUM:
the x0 residual is seeded by an alpha*I matmul and (1-alpha) is folded
into the edge norms. The layer matrix M_l = (1-beta_l)I + beta_l W_l is
applied as one matmul per 128-node window (lhsT = M_l^T from convT).

h is carried in fp16 and stored to HBM as PAIRS of node rows (one 256B
row = 2 nodes x 64 feats), so each edge's dma_gather descriptor moves a
256B pair-row and descriptor count is halved vs one-row-per-edge. The
edge's node parity is encoded into the one-hot: dstw_pair = dst + 128 *
parity compared against a 256-wide iota builds the even-masked and
odd-masked one-hots in ONE vector op; two matmuls per tile (lhsT = the
even / odd feature half of the gathered pair tile) scatter them.

Each core's h shard all-gathers in two halves (A: local nodes 0..3199,
B: 3200..6249) so the A-AllGather overlaps the tail chunks of a layer.
"""
import numpy as np
import concourse.bass as bass
import concourse.mybir as mybir
from concourse import bacc, tile
from concourse.bass_utils import run_bass_kernel_spmd

mdt = mybir.dt

N = 50000
E = 800000
FIN = 128
HID = 64
L = 8
ALPHA = 0.1
THETA = 0.5
NCORES = 8
NS = N // NCORES            # 6250 nodes per core
NW = (NS + 127) // 128      # 49 windows per core
NSPAD = NW * 128            # 6272
CW = 7                      # windows per chunk
CHUNKS = NW // CW           # 7 chunks
assert CW * CHUNKS == NW
NSA = 3200                  # local nodes in half A (windows 0..24)
NSB = NS - NSA              # 3050 (windows 25..48)
NA = NCORES * NSA           # 25600 rows in h_fullA
NB = NCORES * NSB           # 24400 rows in h_fullB


def _preprocess(x, edge_index, w_in, b_in, conv_w, w_out, b_out):
    row = np.asarray(edge_index[0], dtype=np.int64)
    col = np.asarray(edge_index[1], dtype=np.int64)
    loops = np.arange(N, dtype=np.int64)
    row = np.concatenate([row, loops])
    col = np.concatenate([col, loops])
    deg = np.bincount(col, minlength=N).astype(np.float32)
    dinv = (1.0 / np.sqrt(deg)).astype(np.float32)
    norm = ((1.0 - ALPHA) * dinv[row] * dinv[col]).astype(np.float32)

    # src -> (half, pair-row, parity)
    cs = row // NS
    isrc = row % NS
    in_a = isrc < NSA
    srow = np.where(in_a, cs * NSA + isrc, cs * NSB + (isrc - NSA))
    spair = srow // 2
    sparity = srow % 2

    per_core = []
    for c in range(NCORES):
        m = (col >= c * NS) & (col < (c + 1) * NS)
        d = col[m] - c * NS
        o = np.argsort(d, kind="stable")
        per_core.append((spair[m][o], sparity[m][o], in_a[m][o], d[o], norm[m][o]))

    counts = np.zeros((NCORES, NW, 2), dtype=np.int64)
    bounds = []
    for c in range(NCORES):
        _, _, a, d, _ = per_core[c]
        wb = np.searchsorted(d, np.arange(0, NSPAD + 1, 128))
        bounds.append(wb)
        for w in range(NW):
            seg = slice(wb[w], wb[w + 1])
            na = int(a[seg].sum())
            counts[c, w] = (na, (wb[w + 1] - wb[w]) - na)
    TAs = np.maximum(np.ceil(counts[:, :, 0].max(axis=0) / 128), 1).astype(np.int64)
    TBs = np.maximum(np.ceil(counts[:, :, 1].max(axis=0) / 128), 1).astype(np.int64)

    gidx_a = np.zeros(NW, dtype=np.int64)
    gidx_b = np.zeros(NW, dtype=np.int64)
    g = 0
    for s in range(CHUNKS):
        for wi in range(CW):
            w = s * CW + wi
            gidx_a[w] = g
            g += TAs[w]
        for wi in range(CW):
            w = s * CW + wi
            gidx_b[w] = g
            g += TBs[w]
    T = g

    # dstw_pair = dst-in-window + 128*parity (fp16-exact); -1 = padding
    dstwp = np.full((NCORES, 128, T), -1.0, dtype=np.float32)
    nrm = np.zeros((NCORES, 128, T), dtype=np.float32)
    idx16 = np.zeros((NCORES, 128, 8 * T), dtype=np.int16)
    for c in range(NCORES):
        r, par, a, d, nv = per_core[c]
        wb = bounds[c]
        for w in range(NW):
            seg = slice(wb[w], wb[w + 1])
            rs, ps_, as_, ds, ns_ = r[seg], par[seg], a[seg], d[seg], nv[seg]
            for p in range(2):
                mask = as_ if p == 0 else ~as_
                rr = rs[mask]
                dd = (ds[mask] - w * 128) + 128.0 * ps_[mask]
                nn = ns_[mask]
                TT = int(TAs[w] if p == 0 else TBs[w])
                g0 = int(gidx_a[w] if p == 0 else gidx_b[w])
                cap = TT * 128
                rrp = np.zeros(cap, np.int64)
                rrp[: len(rr)] = rr
                ddp = np.full(cap, -1.0, np.float32)
                ddp[: len(dd)] = dd
                nnp = np.zeros(cap, np.float32)
                nnp[: len(nn)] = nn
                for t in range(TT):
                    gg = g0 + t
                    dstwp[c, :, gg] = ddp[t * 128 : (t + 1) * 128]
                    nrm[c, :, gg] = nnp[t * 128 : (t + 1) * 128]
                    v = rrp[t * 128 : (t + 1) * 128].astype(np.int16)
                    idx16[c, :, 8 * gg : 8 * (gg + 1)] = np.tile(
                        v.reshape(8, 16).T, (8, 1)
                    )

    w_in = np.asarray(w_in, np.float32)
    conv_w = np.asarray(conv_w, np.float32)
    w_out = np.asarray(w_out, np.float32)
    b_in = np.asarray(b_in, np.float32)
    b_out = np.asarray(b_out, np.float32)
    betas = np.log(THETA / np.arange(1, L + 1, dtype=np.float32) + 1.0)
    convT = np.concatenate(
        [
            ((1.0 - betas[l]) * np.eye(HID, dtype=np.float32) + betas[l] * conv_w[l]).T
            for l in range(L)
        ],
        axis=1,
    )  # [64, L*64]; column block l = M_l^T
    iota2 = np.tile(np.arange(256, dtype=np.float16), (128, 1))
    consts = {
        "w_inT": np.ascontiguousarray(w_in.T),              # [128, 64] f32
        "convT": np.ascontiguousarray(convT),               # [64, 512] f32
        "w_outT16": np.ascontiguousarray(w_out.T.astype(np.float16)),
        "alphaI": np.ascontiguousarray(ALPHA * np.eye(HID, dtype=np.float32)),
        "b_in_col": np.ascontiguousarray(b_in[:, None]),    # [64, 1] f32
        "b_out_col": np.ascontiguousarray(b_out[:, None]),  # [64, 1] f32
        "iota2": np.ascontiguousarray(iota2),               # [128, 256] f16
        "ident16": np.eye(128, dtype=np.float16),
    }

    x = np.asarray(x, np.float32)
    in_maps = []
    for c in range(NCORES):
        xT = np.zeros((FIN, NSPAD), np.float32)
        xT[:, :NS] = x[c * NS : (c + 1) * NS].T
        in_maps.append(
            dict(
                consts,
                xT=np.ascontiguousarray(xT),
                idx16=np.ascontiguousarray(idx16[c]),
                dstwp=np.ascontiguousarray(dstwp[c]),
                nrm=np.ascontiguousarray(nrm[c]),
            )
        )
    meta = (TAs, TBs, gidx_a, gidx_b, T)
    return in_maps, meta


def _build(meta, reps=1, sim_single=False,
           skip_gather=False, skip_scatter=False, local_ag=False,
           nsplit=8, single_packet=False):
    TAs, TBs, gidx_a, gidx_b, T = meta
    sim_single = sim_single or local_ag
    nc = bacc.Bacc(None, target_bir_lowering=False, num_devices=NCORES,
                   num_swdge_queues=4)

    f16 = mdt.float16
    xT_in = nc.dram_tensor("xT", [FIN, NSPAD], mdt.float32, kind="ExternalInput")
    idx_in = nc.dram_tensor("idx16", [128, 8 * T], mdt.int16, kind="ExternalInput")
    dstwp_in = nc.dram_tensor("dstwp", [128, T], mdt.float32, kind="ExternalInput")
    nrm_in = nc.dram_tensor("nrm", [128, T], mdt.float32, kind="ExternalInput")
    w_inT_in = nc.dram_tensor("w_inT", [FIN, HID], mdt.float32, kind="ExternalInput")
    convT_in = nc.dram_tensor("convT", [HID, L * HID], mdt.float32, kind="ExternalInput")
    w_outT16_in = nc.dram_tensor("w_outT16", [HID, HID], f16, kind="ExternalInput")
    alphaI_in = nc.dram_tensor("alphaI", [HID, HID], mdt.float32, kind="ExternalInput")
    b_in_in = nc.dram_tensor("b_in_col", [HID, 1], mdt.float32, kind="ExternalInput")
    b_out_in = nc.dram_tensor("b_out_col", [HID, 1], mdt.float32, kind="ExternalInput")
    iota2_in = nc.dram_tensor("iota2", [128, 256], f16, kind="ExternalInput")
    ident16_in = nc.dram_tensor("ident16", [128, 128], f16, kind="ExternalInput")

    out_t = nc.dram_tensor("out", [NS, HID], mdt.float32, kind="ExternalOutput")

    bounceA = nc.dram_tensor("bounceA", [NSA, HID], f16)
    bounceB = nc.dram_tensor("bounceB", [NSB, HID], f16)
    h_fullA = nc.dram_tensor("h_fullA", [NA, HID], f16, addr_space="Shared")
    h_fullB = nc.dram_tensor("h_fullB", [NB, HID], f16, addr_space="Shared")

    NLA = [int(TAs[s * CW : (s + 1) * CW].sum()) for s in range(CHUNKS)]
    NLB = [int(TBs[s * CW : (s + 1) * CW].sum()) for s in range(CHUNKS)]
    MAXA, MAXB = max(NLA), max(NLB)

    with tile.TileContext(nc) as tc, \
         tc.tile_pool(name="const", bufs=1) as cpool, \
         tc.tile_pool(name="gath", bufs=2) as gpool, \
         tc.tile_pool(name="oh", bufs=8) as ohpool, \
         tc.tile_pool(name="zt", bufs=2) as zpool, \
         tc.tile_pool(name="hst", bufs=2) as hstpool, \
         tc.tile_pool(name="ps_agg", bufs=2, space="PSUM") as psum_agg, \
         tc.tile_pool(name="ps_mm", bufs=1, space="PSUM") as psum_mm, \
         tc.tile_pool(name="ps_tr", bufs=2, space="PSUM") as psum_tr:

        iota2_t = cpool.tile([128, 256], f16)
        nc.sync.dma_start(iota2_t[:], iota2_in[:])
        ident16_t = cpool.tile([128, 128], f16)
        nc.sync.dma_start(ident16_t[:], ident16_in[:])
        w_inT_t = cpool.tile([FIN, HID], mdt.float32)
        nc.sync.dma_start(w_inT_t[:], w_inT_in[:])
        convT_t = cpool.tile([HID, L * HID], mdt.float32)
        nc.sync.dma_start(convT_t[:], convT_in[:])
        w_outT16_t = cpool.tile([HID, HID], f16)
        nc.sync.dma_start(w_outT16_t[:], w_outT16_in[:])
        alphaI_t = cpool.tile([HID, HID], mdt.float32)
        nc.sync.dma_start(alphaI_t[:], alphaI_in[:])
        b_in_t = cpool.tile([HID, 1], mdt.float32)
        nc.sync.dma_start(b_in_t[:], b_in_in[:])
        b_out_t = cpool.tile([HID, 1], mdt.float32)
        nc.sync.dma_start(b_out_t[:], b_out_in[:])
        idx_t = cpool.tile([128, 8 * T], mdt.int16)
        nc.sync.dma_start(idx_t[:], idx_in[:])
        dstwp_t = cpool.tile([128, T], mdt.float32)
        nc.sync.dma_start(dstwp_t[:], dstwp_in[:])
        nrm_t = cpool.tile([128, T], mdt.float32)
        nc.sync.dma_start(nrm_t[:], nrm_in[:])
        xT_t = cpool.tile([FIN, NSPAD], mdt.float32)
        nc.sync.dma_start(xT_t[:], xT_in[:])

        h_sbT = cpool.tile([HID, NSPAD], f16)           # h^T, fp16
        x0T = cpool.tile([HID, NSPAD], mdt.float32)     # h0^T, fp32

        if skip_gather:
            gA_c = cpool.tile([128, MAXA, 2 * HID], f16)
            gB_c = cpool.tile([128, MAXB, 2 * HID], f16)
            nc.gpsimd.memset(gA_c[:], 0.0)
            nc.gpsimd.memset(gB_c[:], 0.0)

        qctr = [0]

        def store_chunk(s, src_sb):
            r0 = s * CW * 128
            for wi in range(CW):
                w = s * CW + wi
                a = w * 128
                b = min(a + 128, NS)
                if a >= NS:
                    break
                seg = src_sb[: b - a, wi * HID : (wi + 1) * HID]
                if b <= NSA:
                    nc.sync.dma_start(bounceA[a:b, :], seg)
                elif a >= NSA:
                    nc.sync.dma_start(bounceB[a - NSA : b - NSA, :], seg)
                else:
                    nc.sync.dma_start(bounceA[a:NSA, :],
                                      src_sb[: NSA - a, wi * HID : (wi + 1) * HID])
                    nc.sync.dma_start(bounceB[: b - NSA, :],
                                      src_sb[NSA - a : b - a, wi * HID : (wi + 1) * HID])

        def transpose_store(s, srcT):
            """srcT: [64, CW*128] fp16 -> [128, CW*64] fp16 -> bounce."""
            ptr = psum_tr.tile([128, CW * HID], f16, tag="tr")
            for wi in range(CW):
                nc.tensor.transpose(
                    ptr[:, wi * HID : (wi + 1) * HID],
                    srcT[:, wi * 128 : (wi + 1) * 128],
                    ident16_t[:HID, :HID],
                )
            hst = hstpool.tile([128, CW * HID], f16, tag="hst")
            nc.scalar.copy(hst[:], ptr[:])
            store_chunk(s, hst)

        def ag(which):
            bounce, h_full, nsh = (
                (bounceA, h_fullA, NSA) if which == 0 else (bounceB, h_fullB, NSB)
            )
            if sim_single:
                nc.sync.dma_start(h_full[:nsh, :], bounce[:])
            else:
                nc.gpsimd.collective_compute(
                    "AllGather", mybir.AluOpType.bypass,
                    replica_groups=[list(range(NCORES))],
                    ins=[bounce[:]], outs=[h_full[:]],
                )

        # pair-row views of h_full for the gather: [rows/2, 128]
        gsrcA = h_fullA[:].rearrange("(a b) c -> a (b c)", b=2)
        gsrcB = h_fullB[:].rearrange("(a b) c -> a (b c)", b=2)

        for rep_i in range(reps):
            # ---- h0^T = relu(w_in @ x^T + b_in); x0T = h0^T; h fp16 ----
            for s in range(CHUNKS):
                ps2 = psum_mm.tile([HID, CW * 128], mdt.float32, tag="mm")
                for wi in range(CW):
                    w = s * CW + wi
                    nc.tensor.matmul(
                        ps2[:, wi * 128 : (wi + 1) * 128],
                        w_inT_t[:], xT_t[:, w * 128 : (w + 1) * 128],
                        start=True, stop=True,
                    )
                sl = slice(s * CW * 128, (s + 1) * CW * 128)
                nc.scalar.activation(h_sbT[:, sl], ps2[:],
                                     mybir.ActivationFunctionType.Relu,
                                     bias=b_in_t[:])
                nc.scalar.activation(x0T[:, sl], ps2[:],
                                     mybir.ActivationFunctionType.Relu,
                                     bias=b_in_t[:])
                transpose_store(s, h_sbT[:, sl])
            ag(0)
            ag(1)

            # ---- layers ----
            for l in range(L):
                for s in range(CHUNKS):
                    na, nb = NLA[s], NLB[s]
                    basea = int(gidx_a[s * CW])
                    baseb = int(gidx_b[s * CW])
                    if skip_gather:
                        gA, gB = gA_c, gB_c
                    else:
                        gA = gpool.tile([128, MAXA, 2 * HID], f16, tag="gA")
                        gB = gpool.tile([128, MAXB, 2 * HID], f16, tag="gB")

                        def one_gather(dst, src_ap, tile0, ntiles):
                            bnds = [tile0 + (ntiles * i) // nsplit
                                    for i in range(nsplit + 1)]
                            for i in range(nsplit):
                                a, b = bnds[i], bnds[i + 1]
                                if a == b:
                                    continue
                                nidx = (b - a) * 128
                                nc.gpsimd.dma_gather(
                                    dst[:, a - tile0 : b - tile0, :], src_ap,
                                    idx_t[:, 8 * a : 8 * b], nidx, nidx, 2 * HID,
                                    single_packet=single_packet,
                                    queue_num=qctr[0] % 4,
                                )
                                qctr[0] += 1

                        one_gather(gA, gsrcA, basea, na)
                        one_gather(gB, gsrcB, baseb, nb)

                    aggt = psum_agg.tile([HID, CW * 128], mdt.float32, tag="agg")
                    for wi in range(CW):
                        w = s * CW + wi
                        oslice = aggt[:, wi * 128 : (wi + 1) * 128]
                        if skip_scatter:
                            nc.tensor.matmul(oslice, alphaI_t[:],
                                             x0T[:, w * 128 : (w + 1) * 128],
                                             start=True, stop=True)
                            continue
                        ntiles = int(TAs[w] + TBs[w])
                        nc.tensor.matmul(oslice, alphaI_t[:],
                                         x0T[:, w * 128 : (w + 1) * 128],
                                         start=True, stop=False)
                        k = 0
                        for p in range(2):
                            TT = int(TAs[w] if p == 0 else TBs[w])
                            g0 = int(gidx_a[w] if p == 0 else gidx_b[w])
                            gsrc = gA if p == 0 else gB
                            gbase = basea if p == 0 else baseb
                            for t in range(TT):
                                gg = g0 + t
                                slot = gg - gbase
                                oh = ohpool.tile([128, 256], f16, tag="oh")
                                nc.vector.tensor_scalar(
                                    oh[:], iota2_t[:],
                                    dstwp_t[:, gg : gg + 1], nrm_t[:, gg : gg + 1],
                                    mybir.AluOpType.is_equal, mybir.AluOpType.mult,
                                )
                                nc.tensor.matmul(
                                    oslice, gsrc[:, slot, 0:HID], oh[:, 0:128],
                                    start=False, stop=False,
                                )
                                nc.tensor.matmul(
                                    oslice, gsrc[:, slot, HID : 2 * HID],
                                    oh[:, 128:256],
                                    start=False, stop=(k == ntiles - 1),
                                )
                                k += 1
                    zt = zpool.tile([HID, CW * 128], mdt.float32, tag="zt")
                    nc.scalar.copy(zt[:], aggt[:])
                    ps2 = psum_mm.tile([HID, CW * 128], mdt.float32, tag="mm")
                    for wi in range(CW):
                        nc.tensor.matmul(
                            ps2[:, wi * 128 : (wi + 1) * 128],
                            convT_t[:, l * HID : (l + 1) * HID],
                            zt[:, wi * 128 : (wi + 1) * 128],
                            start=True, stop=True,
                        )
                    sl = slice(s * CW * 128, (s + 1) * CW * 128)
                    nc.scalar.activation(h_sbT[:, sl], ps2[:],
                                         mybir.ActivationFunctionType.Relu)
                    if l < L - 1:
                        transpose_store(s, h_sbT[:, sl])
                if l < L - 1:
                    ag(0)
                    ag(1)

            # ---- out^T = w_out @ h^T + b_out (fp16 path, cast-DMA out) ----
            for s in range(CHUNKS):
                ps2 = psum_mm.tile([HID, CW * 128], mdt.float32, tag="mm")
                for wi in range(CW):
                    nc.tensor.matmul(
                        ps2[:, wi * 128 : (wi + 1) * 128],
                        w_outT16_t[:],
                        h_sbT[:, (s * CW + wi) * 128 : (s * CW + wi + 1) * 128],
                        start=True, stop=True,
                    )
                ot = zpool.tile([HID, CW * 128], f16, tag="ot")
                nc.vector.tensor_scalar_add(ot[:], ps2[:], b_out_t[:])
                ptr = psum_tr.tile([128, CW * HID], f16, tag="tr")
                for wi in range(CW):
                    nc.tensor.transpose(
                        ptr[:, wi * HID : (wi + 1) * HID],
                        ot[:, wi * 128 : (wi + 1) * 128],
                        ident16_t[:HID, :HID],
                    )
                hst = hstpool.tile([128, CW * HID], f16, tag="hst")
                nc.scalar.copy(hst[:], ptr[:])
                for wi in range(CW):
                    w = s * CW + wi
                    a = w * 128
                    b = min(a + 128, NS)
                    if a >= NS:
                        break
                    nc.gpsimd.dma_start(out_t[a:b, :],
                                        hst[: b - a, wi * HID : (wi + 1) * HID])

    nc.finalize()
    return nc


def kernel(**inputs) -> np.ndarray:
    in_maps, meta = _preprocess(
        inputs["x"], inputs["edge_index"], inputs["w_in"], inputs["b_in"],
        inputs["conv_w"], inputs["w_out"], inputs["b_out"],
    )
    nc = _build(meta)
    res = run_bass_kernel_spmd(nc, in_maps, list(range(NCORES)))
    out = np.concatenate([res.results[c]["out"] for c in range(NCORES)], axis=0)
    return out
